# revision 24
# baseline (speedup 1.0000x reference)
"""MDTA (Restormer channel attention) Bass/Tile kernel for 8 Trainium2 cores.

Sharding: spatial. Core c handles batch b=c//4, image rows 64*(c%4) .. +64.
The channel attention Gram G = Q K^T and the L2 norms are sums over spatial
positions, so each core accumulates per-head joint [q|k] 96x96 Gram partials
locally and one tiny (2 x 96 x 768 fp32) AllReduce combines them; the
normalization (F.normalize) is applied afterwards as row/col scaling of G.
Everything else (qkv 1x1 conv GEMM, depthwise 3x3, softmax, attn apply,
project_out GEMM) is core-local.

Device channel layout (host pre-permutes all weights to match):
  o-tiles 0..5  (768 ch): per head h: [q_h (48) | k_h (48)] interleaved -> the
                joint per-head Gram block is contiguous, and its diagonal
                gives ssq/ssk for the L2 norms.
  o-tiles 6..9  (512 ch): v padded to 64 per head (48 real + 16 zero rows) so
                heads never straddle 128-partition tile boundaries in the
                attention-apply matmul.
"""

import sys

for p in ("/opt/trn_rl_repo", "/opt/pypackages"):
    if p not in sys.path:
        sys.path.insert(0, p)

import numpy as np
import ml_dtypes

import concourse.bass as bass
import concourse.mybir as mybir
import concourse.tile as tile
import concourse.bacc as bacc
from concourse.bass_utils import run_bass_kernel_spmd

BF16 = ml_dtypes.bfloat16

B, DIM, HGT, WID = 2, 384, 256, 256
HEADS = 8
HD = DIM // HEADS  # 48
N_CORES = 8
ROWS = HGT // 4  # 64 output rows per core
WP = WID + 2  # 258 padded width
HP = ROWS + 2  # 66 padded rows per core
NBLK = 8  # row super-blocks per core
BR = ROWS // NBLK  # 8 output rows per block
BP = BR + 2  # 10 padded rows per block
BN = BR * WID  # 2048 output cols per block
NLOC = ROWS * WID  # 16384 output cols per core

CQK = 2 * DIM  # 768 interleaved q/k channels
CV = HEADS * 64  # 512 padded v channels
COUT = CQK + CV  # 1280 total device channels
OT_QK = CQK // 128  # 6
OT_V = CV // 128  # 4
OT = OT_QK + OT_V  # 10
CT = DIM // 128  # 3 x c-tiles
GSTRIDE = WP * BP  # 2580 cols per block GEMM
GCH = 430  # GEMM moving chunk (6*430 = 2580)

F32 = mybir.dt.float32
BF = mybir.dt.bfloat16


def _build_program():
    nc = bacc.Bacc(
        "TRN2",
        target_bir_lowering=False,
        debug=False,
        num_devices=N_CORES,
    )

    xp = nc.dram_tensor("xp", [CT, 128, HP * WP], BF, kind="ExternalInput")
    wqkvT = nc.dram_tensor("wqkvT", [CT, 128, COUT], BF, kind="ExternalInput")
    wdw = nc.dram_tensor("wdw", [OT, 128, 9], F32, kind="ExternalInput")
    wprojT = nc.dram_tensor("wprojT", [OT_V, 128, DIM], BF, kind="ExternalInput")
    tempb = nc.dram_tensor("tempb", [HD, HEADS], F32, kind="ExternalInput")
    eyeb = nc.dram_tensor("eyeb", [128, 128], BF, kind="ExternalInput")
    eyem = nc.dram_tensor("eyem", [96, 768], F32, kind="ExternalInput")
    mask8 = nc.dram_tensor("mask8", [HEADS, DIM], BF, kind="ExternalInput")
    wdiag = nc.dram_tensor("wdiag", [OT, 128, 4 * 128], BF, kind="ExternalInput")
    y = nc.dram_tensor("y", [CT, 128, NLOC], F32, kind="ExternalOutput")

    AOP = mybir.AluOpType
    ACT = mybir.ActivationFunctionType

    with tile.TileContext(nc) as tc:
        with (
            tc.tile_pool(name="const", bufs=1) as constp,
            tc.tile_pool(name="xin", bufs=1) as xinp,
            tc.tile_pool(name="pre", bufs=1) as prep,
            tc.tile_pool(name="acc", bufs=1) as accp,
            tc.tile_pool(name="qkc", bufs=1) as qkcp,
            tc.tile_pool(name="vc", bufs=2) as vcp,
            tc.tile_pool(name="qkt", bufs=2) as qktp,
            tc.tile_pool(name="small", bufs=1) as smallp,
            tc.tile_pool(name="vin", bufs=2) as vinp,
            tc.tile_pool(name="att", bufs=2) as attp,
            tc.tile_pool(name="yout", bufs=2) as youtp,
            tc.tile_pool(name="psA", bufs=2, space="PSUM") as psA,
            tc.tile_pool(name="psC", bufs=4, space="PSUM") as psC,
            tc.tile_pool(name="psG", bufs=1, space="PSUM") as psG,
            tc.tile_pool(name="dram", bufs=1, space="DRAM") as dramp,
        ):
            # ---- resident constants --------------------------------------
            wq_sb = []
            for ct in range(CT):
                t = constp.tile([128, COUT], BF, tag=f"wq{ct}")
                nc.sync.dma_start(t[:], wqkvT[ct])
                wq_sb.append(t)
            wdw_sb = []
            for ot in range(OT):
                t = constp.tile([128, 9], F32, tag=f"wdw{ot}")
                nc.sync.dma_start(t[:], wdw[ot])
                wdw_sb.append(t)
            wp_sb = []
            for vt in range(OT_V):
                t = constp.tile([128, DIM], BF, tag=f"wp{vt}")
                nc.sync.dma_start(t[:], wprojT[vt])
                wp_sb.append(t)
            tempb_sb = constp.tile([HD, HEADS], F32, tag="tempb")
            nc.sync.dma_start(tempb_sb[:], tempb[:])
            eyeb_sb = constp.tile([128, 128], BF, tag="eyeb")
            nc.sync.dma_start(eyeb_sb[:], eyeb[:])
            eyem_sb = constp.tile([96, 768], F32, tag="eyem")
            nc.sync.dma_start(eyem_sb[:], eyem[:])
            ones_sb = constp.tile([HEADS, HD], BF, tag="ones")
            nc.vector.memset(ones_sb[:], 1.0)
            mask8_sb = constp.tile([HEADS, DIM], BF, tag="mask8")
            nc.sync.dma_start(mask8_sb[:], mask8[:])
            wdiag_sb = []
            for ot in range(OT):
                t = constp.tile([128, 4 * 128], BF, tag=f"wdiag{ot}")
                nc.sync.dma_start(t[:], wdiag[ot])
                wdiag_sb.append(t)

            v_dram = dramp.tile([OT_V, 128, NLOC], BF)
            qk_dram = dramp.tile([OT_QK, 128, NLOC], BF)
            cc_in = dramp.tile([96, 768], F32)
            cc_out = dramp.tile([96, 768], F32)

            # Gram accumulators: 2 banks x [96, 4*96] (4 heads per bank)
            gram_ps = [
                psG.tile([96, 384], F32, tag=f"g{i}", name=f"gram{i}")
                for i in range(2)
            ]

            # ---- phase 1: stream row blocks ------------------------------
            for k in range(NBLK):
                # x rows 8k .. 8k+10 (padded indexing), all 3 c-tiles
                x_sb = []
                for ct in range(CT):
                    t = xinp.tile([128, GSTRIDE], BF, tag=f"x{ct}")
                    nc.sync.dma_start(
                        t[:], xp[ct][:, k * BR * WP : k * BR * WP + GSTRIDE]
                    )
                    x_sb.append(t)

                # qkv pointwise GEMM for the block
                pre_sb = []
                for ot in range(OT):
                    t = prep.tile([128, GSTRIDE], BF, tag=f"pre{ot}")
                    pre_sb.append(t)
                for ot in range(OT):
                    for g in range(GSTRIDE // GCH):
                        ps = psA.tile([128, 512], F32, tag="gemm")
                        for ct in range(CT):
                            nc.tensor.matmul(
                                ps[:, :GCH],
                                lhsT=wq_sb[ct][:, ot * 128 : (ot + 1) * 128],
                                rhs=x_sb[ct][:, g * GCH : (g + 1) * GCH],
                                start=(ct == 0),
                                stop=(ct == CT - 1),
                            )
                        nc.scalar.copy(
                            pre_sb[ot][:, g * GCH : (g + 1) * GCH], ps[:, :GCH]
                        )

                # depthwise 3x3: taps 0-4 on DVE (tensor_scalar +
                # tensor_tensor), taps 5-8 as diag-matmul PSUM accumulation
                # on PE with shifted moving-operand APs; merged per 512-chunk
                for ot in range(OT):
                    pre_r = pre_sb[ot][:].rearrange("p (r w) -> p r w", w=WP)
                    if ot < OT_QK:
                        dst = qkcp.tile([128, BN], BF, tag=f"qk{ot}")
                    else:
                        dst = vcp.tile([128, BN], BF, tag=f"v{ot - OT_QK}")
                    accA = accp.tile([128, BN], BF, tag="accA")
                    accB = accp.tile([128, BN], BF, tag="accB")
                    tmp = accp.tile([128, BN], BF, tag="tmp")
                    pp = [accA, accB]
                    for s in range(5):
                        dh, dw = s // 3, s % 3
                        srcap = pre_r[:, dh : dh + BR, dw : dw + WID]
                        wcol = wdw_sb[ot][:, s : s + 1]
                        cur, nxt = pp[(s + 1) % 2], pp[s % 2]
                        if s == 0:
                            nc.vector.tensor_scalar_mul(nxt[:], srcap, wcol)
                        else:
                            nc.vector.tensor_scalar_mul(tmp[:], srcap, wcol)
                            nc.vector.tensor_tensor(
                                nxt[:], cur[:], tmp[:], AOP.add
                            )
                    acc5 = pp[0]  # after s=0..4 the result is in accA
                    for g in range(4):
                        pc = psC.tile([128, 512], F32, tag="conv")
                        for i, s in enumerate((5, 6, 7, 8)):
                            dh, dw = s // 3, s % 3
                            rhs = pre_r[
                                :, dh + 2 * g : dh + 2 * g + 2, dw : dw + WID
                            ]
                            nc.tensor.matmul(
                                pc[:],
                                lhsT=wdiag_sb[ot][:, i * 128 : (i + 1) * 128],
                                rhs=rhs,
                                start=(i == 0),
                                stop=(i == 3),
                            )
                        nc.vector.tensor_tensor(
                            dst[:, g * 512 : (g + 1) * 512],
                            acc5[:, g * 512 : (g + 1) * 512],
                            pc[:],
                            AOP.add,
                        )
                    if ot >= OT_QK:
                        vt = ot - OT_QK
                        nc.sync.dma_start(
                            v_dram[vt][:, k * BN : (k + 1) * BN], dst[:]
                        )
                    else:
                        nc.sync.dma_start(
                            qk_dram[ot][:, k * BN : (k + 1) * BN], dst[:]
                        )

                # xbar-transpose q/k n-tiles from DRAM, accumulate Grams
                qkd2 = qk_dram[:, :, :].rearrange("t p n -> (t p) n")
                for half in range(BN // 128):
                    nt = k * (BN // 128) + half
                    qkT = qktp.tile([128, CQK], BF, tag="qkT")
                    teng = nc.sync if half % 2 == 0 else nc.scalar
                    teng.dma_start_transpose(
                        qkT[:], qkd2[:, nt * 128 : (nt + 1) * 128]
                    )
                    first = k == 0 and half == 0
                    last = k == NBLK - 1 and half == BN // 128 - 1
                    for h in range(HEADS):
                        nc.tensor.matmul(
                            gram_ps[h // 4][:, (h % 4) * 96 : (h % 4) * 96 + 96],
                            lhsT=qkT[:, h * 96 : h * 96 + 96],
                            rhs=qkT[:, h * 96 : h * 96 + 96],
                            start=first,
                            stop=last,
                            skip_group_check=True,
                        )

            # ---- phase 1.5: per-batch-group AllReduce --------------------
            ccin_sb = smallp.tile([96, 768], F32, tag="ccin")
            for g in range(2):
                nc.scalar.copy(
                    ccin_sb[:, g * 384 : (g + 1) * 384], gram_ps[g][:]
                )
            nc.sync.dma_start(cc_in[:], ccin_sb[:])
            nc.gpsimd.collective_compute(
                "AllReduce",
                AOP.add,
                replica_groups=[[0, 1, 2, 3], [4, 5, 6, 7]],
                ins=[cc_in.opt()],
                outs=[cc_out.opt()],
            )
            gred = smallp.tile([96, 768], F32, tag="gred")
            nc.sync.dma_start(gred[:], cc_out[:])

            # ---- phase 2: norms, scaling, softmax, A^T -------------------
            # diag -> per-channel sum of squares [96(joint c), 8(head)]
            dm = smallp.tile([96, 768], F32, tag="dm")
            nc.vector.tensor_tensor(dm[:], gred[:], eyem_sb[:], AOP.mult)
            dsum = smallp.tile([96, HEADS], F32, tag="dsum")
            nc.vector.tensor_reduce(
                dsum[:],
                dm[:].rearrange("p (h d) -> p h d", d=96),
                axis=mybir.AxisListType.X,
                op=AOP.add,
            )
            norms = smallp.tile([96, HEADS], F32, tag="norms")
            nc.scalar.sqrt(norms[:], dsum[:])
            nc.vector.tensor_scalar_max(norms[:], norms[:], 1e-12)
            rsc = smallp.tile([96, HEADS], F32, tag="rsc")
            nc.vector.reciprocal(rsc[:], norms[:])

            # rk broadcast [48, h*48+d] = rsc[48+d, h]:
            # transpose rsc -> rscT [8, 96]; rkrep[h', (h,d)] = rscT[h', 48+d]
            # masked by delta(h'=h); then ones[8,48].T @ rkrep sums out h'.
            rscb = smallp.tile([96, HEADS], BF, tag="rscb")
            nc.vector.tensor_copy(rscb[:], rsc[:])
            rscT_ps = psC.tile([128, 128], BF, tag="conv")
            nc.tensor.transpose(
                rscT_ps[:HEADS, :96], rscb[:], eyeb_sb[:96, :96]
            )
            rscT = smallp.tile([HEADS, 96], BF, tag="rscT")
            nc.vector.tensor_copy(rscT[:], rscT_ps[:HEADS, :96])
            rkrep = smallp.tile([HEADS, DIM], BF, tag="rkrep")
            mask3d = mask8_sb[:].rearrange("p (h d) -> p h d", d=HD)
            rk3d = rscT[:, HD : 2 * HD].rearrange("p (o d) -> p o d", o=1)
            mask3d, rk3d = bass.broadcast_tensor_aps(mask3d, rk3d)
            nc.vector.tensor_tensor(
                rkrep[:].rearrange("p (h d) -> p h d", d=HD),
                mask3d,
                rk3d,
                AOP.mult,
            )
            rkb_ps = psA.tile([128, 512], F32, tag="gemm")
            nc.tensor.matmul(
                rkb_ps[:HD, :DIM],
                lhsT=ones_sb[:],
                rhs=rkrep[:],
                start=True,
                stop=True,
            )
            # logits L[c, h, d] = G_qk * rk * (temp_h * rq)
            L = smallp.tile([HD, DIM], F32, tag="L")
            gqk = gred[0:HD].rearrange("p (h d) -> p h d", d=96)[:, :, HD : 2 * HD]
            nc.vector.tensor_tensor(
                L[:].rearrange("p (h d) -> p h d", d=HD),
                gqk,
                rkb_ps[:HD, :DIM].rearrange("p (h d) -> p h d", d=HD),
                AOP.mult,
            )
            tsc = smallp.tile([HD, HEADS], F32, tag="tsc")
            nc.vector.tensor_tensor(tsc[:], tempb_sb[:], rsc[0:HD, :], AOP.mult)
            for h in range(HEADS):
                nc.vector.tensor_scalar_mul(
                    L[:, h * HD : (h + 1) * HD],
                    L[:, h * HD : (h + 1) * HD],
                    tsc[:, h : h + 1],
                )
            # softmax over d (free dim, per 48-block)
            mx = smallp.tile([HD, HEADS], F32, tag="mx")
            nc.vector.tensor_reduce(
                mx[:],
                L[:].rearrange("p (h d) -> p h d", d=HD),
                axis=mybir.AxisListType.X,
                op=AOP.max,
            )
            for h in range(HEADS):
                nc.vector.tensor_scalar_sub(
                    L[:, h * HD : (h + 1) * HD],
                    L[:, h * HD : (h + 1) * HD],
                    mx[:, h : h + 1],
                )
            nc.scalar.activation(L[:], L[:], ACT.Exp)
            sm = smallp.tile([HD, HEADS], F32, tag="sm")
            nc.vector.tensor_reduce(
                sm[:],
                L[:].rearrange("p (h d) -> p h d", d=HD),
                axis=mybir.AxisListType.X,
                op=AOP.add,
            )
            rs = smallp.tile([HD, HEADS], F32, tag="rs")
            nc.vector.reciprocal(rs[:], sm[:])
            for h in range(HEADS):
                nc.vector.tensor_scalar_mul(
                    L[:, h * HD : (h + 1) * HD],
                    L[:, h * HD : (h + 1) * HD],
                    rs[:, h : h + 1],
                )
            Lb = smallp.tile([HD, DIM], BF, tag="Lb")
            nc.vector.tensor_copy(Lb[:], L[:])

            # A^T tiles: per v-tile [128, 128] block-diag (2 heads at 0/64)
            atb = []
            for t in range(OT_V):
                ps = psC.tile([128, 128], BF, tag="conv")
                for i in range(2):
                    h = 2 * t + i
                    nc.tensor.transpose(
                        ps[i * 64 : i * 64 + HD, i * 64 : i * 64 + HD],
                        Lb[:, h * HD : (h + 1) * HD],
                        eyeb_sb[:HD, :HD],
                    )
                a = smallp.tile([128, 128], BF, tag=f"atb{t}")
                nc.vector.memset(a[:], 0.0)
                for i in range(2):
                    nc.vector.tensor_copy(
                        a[i * 64 : i * 64 + HD, i * 64 : i * 64 + HD],
                        ps[i * 64 : i * 64 + HD, i * 64 : i * 64 + HD],
                    )
                atb.append(a)

            # ---- phase 3: apply + project_out (proj pipelined 1 nt) ------
            VCH = 1024  # v reload chunk

            def _proj(nt, att_sb):
                for po in range(CT):
                    ps = psA.tile([128, 512], F32, tag="gemm", name=f"proj{nt}_{po}")
                    for t in range(OT_V):
                        nc.tensor.matmul(
                            ps[:],
                            lhsT=wp_sb[t][:, po * 128 : (po + 1) * 128],
                            rhs=att_sb[t][:],
                            start=(t == 0),
                            stop=(t == OT_V - 1),
                        )
                    ysb = youtp.tile([128, 512], F32, tag="ysb", name=f"y{nt}_{po}")
                    nc.scalar.copy(ysb[:], ps[:])
                    nc.sync.dma_start(
                        y[po][:, nt * 512 : (nt + 1) * 512], ysb[:]
                    )

            prev = None
            for nt in range(NLOC // 512):
                if nt % (VCH // 512) == 0:
                    v_sb = []
                    for t in range(OT_V):
                        vt_ = vinp.tile([128, VCH], BF, tag=f"vin{t}")
                        nc.sync.dma_start(
                            vt_[:],
                            v_dram[t][:, nt * 512 : nt * 512 + VCH],
                        )
                        v_sb.append(vt_)
                off = (nt % (VCH // 512)) * 512
                att_sb = []
                for t in range(OT_V):
                    ps = psA.tile([128, 512], F32, tag="gemm", name=f"app{nt}_{t}")
                    nc.tensor.matmul(
                        ps[:],
                        lhsT=atb[t][:],
                        rhs=v_sb[t][:, off : off + 512],
                        start=True,
                        stop=True,
                    )
                    a = attp.tile([128, 512], BF, tag=f"att{t}")
                    nc.scalar.copy(a[:], ps[:])
                    att_sb.append(a)
                if prev is not None:
                    _proj(nt - 1, prev)
                prev = att_sb
            _proj(NLOC // 512 - 1, prev)

    nc.compile()
    return nc


_NC = None


def _get_program():
    global _NC
    if _NC is None:
        _NC = _build_program()
    return _NC


def _prep_weights(qkv_w, dw_w, proj_w, log_temp):
    """Host-side weight permutation/padding. Returns dict of shared inputs."""
    qkv_w = np.asarray(qkv_w, np.float32)
    dw_w = np.asarray(dw_w, np.float32).reshape(3 * DIM, 9)
    proj_w = np.asarray(proj_w, np.float32)
    temp = np.log1p(np.exp(np.asarray(log_temp, np.float32).reshape(HEADS)))
    temp = temp + 1e-6

    # permutation: first 768 = per head [q_h | k_h]; then v padded to 64/head
    perm_qk = np.concatenate(
        [
            np.concatenate([np.arange(h * HD, (h + 1) * HD),
                            DIM + np.arange(h * HD, (h + 1) * HD)])
            for h in range(HEADS)
        ]
    )
    wq = np.zeros((COUT, DIM), np.float32)
    wd = np.zeros((COUT, 9), np.float32)
    wq[:CQK] = qkv_w[perm_qk]
    wd[:CQK] = dw_w[perm_qk]
    for h in range(HEADS):
        src = 2 * DIM + h * HD
        dst = CQK + h * 64
        wq[dst : dst + HD] = qkv_w[src : src + HD]
        wd[dst : dst + HD] = dw_w[src : src + HD]

    wqkvT = np.ascontiguousarray(wq.T.reshape(CT, 128, COUT)).astype(BF16)
    wdw = np.ascontiguousarray(wd.reshape(OT, 128, 9))

    wpad = np.zeros((CV, DIM), np.float32)
    for h in range(HEADS):
        wpad[h * 64 : h * 64 + HD] = proj_w[:, h * HD : (h + 1) * HD].T
    wprojT = np.ascontiguousarray(wpad.reshape(OT_V, 128, DIM)).astype(BF16)

    tempb = np.broadcast_to(temp[None, :], (HD, HEADS)).copy()
    eyeb = np.eye(128, dtype=np.float32).astype(BF16)
    eyem = np.tile(np.eye(96, dtype=np.float32), (1, 8)).copy()
    mask8 = np.repeat(np.eye(HEADS, dtype=np.float32), HD, axis=1).astype(BF16)
    wdiag = np.zeros((OT, 4, 128, 128), np.float32)
    for ot in range(OT):
        for i, s in enumerate((5, 6, 7, 8)):
            np.fill_diagonal(wdiag[ot, i], wd[ot * 128 : (ot + 1) * 128, s])
    # sbuf layout: [128 part(k), 4*128 free(s, m)]
    wdiag = np.ascontiguousarray(wdiag.transpose(0, 2, 1, 3)).reshape(
        OT, 128, 4 * 128
    ).astype(BF16)
    return {
        "wqkvT": wqkvT,
        "wdw": wdw,
        "wprojT": wprojT,
        "tempb": tempb,
        "eyeb": eyeb,
        "eyem": eyem,
        "mask8": mask8,
        "wdiag": wdiag,
    }


def _prep_x(x):
    """Per-core padded x chunks, bf16: [CT, 128, HP*WP] each."""
    x = np.asarray(x, np.float32)
    chunks = []
    for c in range(N_CORES):
        b, r0 = c // 4, ROWS * (c % 4)
        buf = np.zeros((DIM, HP, WP), np.float32)
        lo, hi = max(r0 - 1, 0), min(r0 + ROWS + 1, HGT)
        buf[:, lo - (r0 - 1) : hi - (r0 - 1), 1 : WID + 1] = x[b, :, lo:hi, :]
        chunks.append(
            np.ascontiguousarray(buf.reshape(CT, 128, HP * WP)).astype(BF16)
        )
    return chunks


def _run(x, qkv_w, dw_w, proj_w, log_temp, trace=False):
    nc = _get_program()
    shared = _prep_weights(qkv_w, dw_w, proj_w, log_temp)
    xchunks = _prep_x(x)
    in_maps = [
        {**shared, "xp": xchunks[c]} for c in range(N_CORES)
    ]
    res = run_bass_kernel_spmd(
        nc, in_maps, core_ids=list(range(N_CORES)), trace=trace
    )
    out = np.empty((B, DIM, HGT, WID), np.float32)
    for c in range(N_CORES):
        b, r0 = c // 4, ROWS * (c % 4)
        yc = res.results[c]["y"].reshape(DIM, ROWS, WID)
        out[b, :, r0 : r0 + ROWS, :] = yc
    return out, res


def kernel(x, qkv_w, dw_w, proj_w, log_temp):
    out, _ = _run(x, qkv_w, dw_w, proj_w, log_temp, trace=False)
    return out
